# revision 1
# baseline (speedup 1.0000x reference)
"""Decoder-only attention kernel for Trainium2 (8 NeuronCores).

Sharding: tensor-parallel over heads (4 groups of 4 heads) x data-parallel
over batch (2), = 8 cores. Each core computes, for its batch b and its 4
heads, the partial output  sum_h (softmax(causal(q_h k_h^T / 8)) v_h) @ O_h
as a [T, D] array; the host sums the 4 partials per batch and adds Ob.

On-core layout strategy ("transposed flash attention"):
  - host supplies xT = x[b].T  [D, T]; QK projections then produce
    qT/kT [dk, T] directly (dk on partitions).
  - scoresT[s, tq] = kT_chunk.T @ qT  (K=dk=64); the two heads of a pair
    sit at base partitions 0/64 so their K=64 matmuls land in disjoint
    PE row groups and overlap in hardware.
  - softmax denominator is accumulated free as a ones-column appended to
    v in the z matmul: zT_aug[65, tq] = [v | 1]^T @ expT  (K=s chunks)
  - v-bias is folded into the v projection as one extra K=1 matmul
    ([x | 1] @ [Wv ; vb]), matching reference semantics exactly.
  - zT [hdk, tq] is exactly the stationary layout the O projection needs.
  - causal: upper-triangle (s,tq) blocks skipped; diagonal blocks
    multiplied post-exp by one of 4 host-precomputed 0/1 masks.
All matmuls run as float32r (full PE rate); PSUM accumulates fp32.
"""

import sys

import numpy as np

if "/opt/trn_rl_repo" not in sys.path:
    sys.path.insert(0, "/opt/trn_rl_repo")

# Model shapes (hardcoded; kernel.py must be self-contained).
B = 2
T = 2048
D = 1024
NH = 16  # total heads
H = 4  # heads per core
DK = 64
NCORES = 8

TQ = 512  # tq chunk (matmul moving free dim)
SK = 128  # s chunk (partition dim of scoresT)

_PROGRAM_CACHE = {}

# test.py can flip these before calling kernel()
TRACE = False
TRACE_KWARGS = {}
USE_FP32R = True
MASK_ENGINE = "vector"  # or "gpsimd"
RB_MODE = "matmul"  # or "gpsimd"


def _build_program(t=T, d=D, reps=1):
    import concourse.bass as bass  # noqa: F401
    import concourse.mybir as mybir
    import concourse.tile as tile
    from concourse import bacc

    f32 = mybir.dt.float32
    mmdt = mybir.dt.float32r if USE_FP32R else mybir.dt.float32

    def r(ap):  # matmul-operand dtype view
        return ap.bitcast(mmdt) if ap.dtype != mmdt else ap

    P = 128
    DC = d // P  # d_model chunks (contraction for projections)
    SC = t // SK  # s chunks
    TC = t // TQ  # tq chunks
    VW = H * (DK + 1)  # v block width per s-chunk: 4 heads x (64 v + 1 ones)
    m = H * DK

    nc = bacc.Bacc(
        "TRN2",
        target_bir_lowering=False,
        debug=False,
        enable_asserts=True,
        num_devices=NCORES,
    )

    xT = nc.dram_tensor("xT", [d, t], mmdt, kind="ExternalInput").ap()
    wq = nc.dram_tensor("wq", [d, m], mmdt, kind="ExternalInput").ap()
    wk = nc.dram_tensor("wk", [d, m], mmdt, kind="ExternalInput").ap()
    wv = nc.dram_tensor("wv", [d, m], mmdt, kind="ExternalInput").ap()
    wo = nc.dram_tensor("wo", [m, d], mmdt, kind="ExternalInput").ap()
    qb = nc.dram_tensor("qb", [P, 2], f32, kind="ExternalInput").ap()
    kb = nc.dram_tensor("kb", [P, 2], f32, kind="ExternalInput").ap()
    vb = nc.dram_tensor("vb", [1, m], mmdt, kind="ExternalInput").ap()
    mask = nc.dram_tensor("mask", [P, P], f32, kind="ExternalInput").ap()
    out = nc.dram_tensor("out", [t, d], f32, kind="ExternalOutput").ap()

    with tile.TileContext(nc) as tc:
        from contextlib import ExitStack

        ctx = ExitStack()
        with ctx:
            const = ctx.enter_context(tc.tile_pool(name="const", bufs=1))

            # ---- resident SBUF tensors ----
            xT_sb = [
                const.tile([P, t], mmdt, tag=f"xT{c}", name=f"xT{c}")
                for c in range(DC)
            ]
            wq_sb = const.tile([P, DC * m], mmdt, tag="wq")
            wk_sb = const.tile([P, DC * m], mmdt, tag="wk")
            wv_sb = const.tile([P, DC * m], mmdt, tag="wv")
            wo_sb = const.tile([P, 2 * d], mmdt, tag="wo")
            qb_sb = const.tile([P, 2], f32, tag="qb")
            kb_sb = const.tile([P, 2], f32, tag="kb")
            vb_sb = const.tile([1, m], mmdt, tag="vb")
            mask_sb = const.tile([P, P], f32, tag="mask")
            ones_dk = const.tile([1, DK], mmdt, tag="ones_dk")
            ones_row = const.tile([1, P], mmdt, tag="ones_row")
            v_sb = const.tile([P, SC * VW], mmdt, tag="v")
            qT_sb = [
                const.tile([P, t], mmdt, tag=f"qT{p}", name=f"qT{p}") for p in range(2)
            ]
            kT_sb = [
                const.tile([P, t], mmdt, tag=f"kT{p}", name=f"kT{p}") for p in range(2)
            ]
            zT_sb = [
                const.tile([P, t], mmdt, tag=f"zT{p}", name=f"zT{p}") for p in range(2)
            ]

            # ---- input DMAs: weights/constants first, then xT streamed in
            # tp-major [128, TQ] pieces so projections start at ~4us
            nc.sync.dma_start(vb_sb[:], vb[:])
            for c in range(DC):
                nc.sync.dma_start(
                    wv_sb[:, c * m : (c + 1) * m], wv[c * P : (c + 1) * P, :]
                )
            for c in range(DC):
                nc.sync.dma_start(
                    xT_sb[c][:, 0:TQ], xT[c * P : (c + 1) * P, 0:TQ]
                )
            for c in range(DC):
                nc.sync.dma_start(
                    wq_sb[:, c * m : (c + 1) * m], wq[c * P : (c + 1) * P, :]
                )
                nc.sync.dma_start(
                    wk_sb[:, c * m : (c + 1) * m], wk[c * P : (c + 1) * P, :]
                )
            nc.sync.dma_start(qb_sb[:], qb[:])
            nc.sync.dma_start(kb_sb[:], kb[:])
            nc.sync.dma_start(mask_sb[:], mask[:])
            if TC > 1:
                for c in range(DC):
                    nc.sync.dma_start(
                        xT_sb[c][:, TQ : 2 * TQ],
                        xT[c * P : (c + 1) * P, TQ : 2 * TQ],
                    )
            for kc in range(2):
                nc.sync.dma_start(
                    wo_sb[:, kc * d : (kc + 1) * d], wo[kc * P : (kc + 1) * P, :]
                )
            for tp in range(2, TC):
                for c in range(DC):
                    nc.sync.dma_start(
                        xT_sb[c][:, tp * TQ : (tp + 1) * TQ],
                        xT[c * P : (c + 1) * P, tp * TQ : (tp + 1) * TQ],
                    )
            ones_stage = const.tile([P, 1], f32, tag="ones_stage")
            nc.vector.memset(ones_stage[:], 1.0)
            nc.vector.tensor_copy(ones_dk[:], ones_stage[0:1, :].to_broadcast([1, DK]))
            nc.vector.tensor_copy(ones_row[:], ones_stage[0:1, :].to_broadcast([1, P]))
            # ones columns of the v blocks (col 64 of each head's 65-wide slot)
            nc.vector.tensor_copy(
                v_sb.rearrange("p (j h c) -> p j h c", h=H, c=DK + 1)[:, :, :, DK :],
                ones_stage[:, None, None, :].to_broadcast([P, SC, H, 1]),
            )

            def _phases():
                mask_tt = (
                    nc.gpsimd.tensor_tensor
                    if MASK_ENGINE == "gpsimd"
                    else nc.vector.tensor_tensor
                )
                with tc.tile_pool(name="pproj", bufs=4, space="PSUM") as pproj:
                    for tp in range(TC):
                        sl = slice(tp * TQ, (tp + 1) * TQ)
                        # ---- v projection (+v bias) for this tp block ----
                        for i in range(4 * tp, 4 * tp + 4):
                            pv = pproj.tile([P, m], f32, tag="mm", name="pv")
                            for c in range(DC):
                                nc.tensor.matmul(
                                    pv[:],
                                    r(xT_sb[c][:, i * P : (i + 1) * P]),
                                    r(wv_sb[:, c * m : (c + 1) * m]),
                                    start=(c == 0),
                                    stop=False,
                                )
                            nc.tensor.matmul(
                                pv[:],
                                r(ones_row[0:1, :]),
                                r(vb_sb[0:1, :]),
                                start=False,
                                stop=True,
                            )
                            nc.vector.tensor_copy(
                                v_sb.rearrange("p (j h c) -> p j h c", h=H, c=DK + 1)[
                                    :, i, :, 0:DK
                                ],
                                pv.rearrange("p (h c) -> p h c", c=DK),
                            )
                        # ---- q/k projections for this tp block ----
                        for p in range(2):
                            pq = pproj.tile([P, TQ], f32, tag="mm", name="pq")
                            for c in range(DC):
                                nc.tensor.matmul(
                                    pq[:],
                                    r(wq_sb[:, c * m + p * P : c * m + (p + 1) * P]),
                                    r(xT_sb[c][:, sl]),
                                    start=(c == 0),
                                    stop=(c == DC - 1),
                                )
                            nc.vector.tensor_scalar_add(
                                qT_sb[p][:, sl], pq[:], qb_sb[:, p : p + 1]
                            )
                            pk = pproj.tile([P, TQ], f32, tag="mm", name="pk")
                            for c in range(DC):
                                nc.tensor.matmul(
                                    pk[:],
                                    r(wk_sb[:, c * m + p * P : c * m + (p + 1) * P]),
                                    r(xT_sb[c][:, sl]),
                                    start=(c == 0),
                                    stop=(c == DC - 1),
                                )
                            nc.vector.tensor_scalar_add(
                                kT_sb[p][:, sl], pk[:], kb_sb[:, p : p + 1]
                            )

                # ---- attention + O projection, tq-chunk outer ----
                with (
                    tc.tile_pool(name="ps", bufs=2, space="PSUM") as ps_pool,
                    tc.tile_pool(name="pz", bufs=4, space="PSUM") as pz_pool,
                    tc.tile_pool(name="po", bufs=2, space="PSUM") as po_pool,
                    tc.tile_pool(name="expt", bufs=8) as exp_pool,
                    tc.tile_pool(name="rcp", bufs=2) as rcp_pool,
                    tc.tile_pool(name="rbs", bufs=3) as rbs_pool,
                    tc.tile_pool(name="osb", bufs=6) as out_pool,
                ):
                    def _o_proj(tq_c):
                        for i in range(4 * tq_c, 4 * tq_c + 4):
                            for d2 in range(d // TQ):
                                po = po_pool.tile([P, TQ], f32, tag="po", name="po")
                                for kc in range(2):
                                    nc.tensor.matmul(
                                        po[:],
                                        r(zT_sb[kc][:, i * P : (i + 1) * P]),
                                        r(
                                            wo_sb[
                                                :,
                                                kc * d
                                                + d2 * TQ : kc * d
                                                + (d2 + 1) * TQ,
                                            ]
                                        ),
                                        start=(kc == 0),
                                        stop=(kc == 1),
                                    )
                                ot = out_pool.tile([P, TQ], f32, tag="osb", name="ot")
                                nc.vector.tensor_copy(ot[:], po[:])
                                nc.sync.dma_start(
                                    out[i * P : (i + 1) * P, d2 * TQ : (d2 + 1) * TQ],
                                    ot[:],
                                )

                    for tcq in range(TC):
                        sl = slice(tcq * TQ, (tcq + 1) * TQ)
                        nsc = min(SC, 4 * tcq + 4)
                        for p in range(2):
                            pz = [
                                pz_pool.tile([DK + 1, TQ], f32, tag="pz", name="pz")
                                for _ in range(2)
                            ]
                            for j in range(nsc):
                                # two heads' K=64 scores matmuls sit at PE row
                                # groups 0-63 / 64-127: issue back-to-back
                                pss, ets = [], []
                                for hh in range(2):
                                    ps = ps_pool.tile([P, TQ], f32, tag="ps", name="ps")
                                    nc.tensor.matmul(
                                        ps[:],
                                        r(
                                            kT_sb[p][
                                                hh * DK : (hh + 1) * DK,
                                                j * SK : (j + 1) * SK,
                                            ]
                                        ),
                                        r(qT_sb[p][hh * DK : (hh + 1) * DK, sl]),
                                        start=True,
                                        stop=True,
                                    )
                                    pss.append(ps)
                                rdiag = j - 4 * tcq
                                for hh in range(2):
                                    et = exp_pool.tile(
                                        [P, TQ], mmdt, tag="expt", name="et"
                                    )
                                    nc.scalar.activation(
                                        et[:],
                                        pss[hh][:],
                                        mybir.ActivationFunctionType.Exp,
                                        scale=0.125,
                                    )
                                    if rdiag >= 0:
                                        # mask only the 128x128 diagonal block
                                        mask_tt(
                                            et[:, rdiag * SK : (rdiag + 1) * SK],
                                            et[:, rdiag * SK : (rdiag + 1) * SK],
                                            mask_sb[:],
                                            op=mybir.AluOpType.mult,
                                        )
                                    ets.append(et)
                                for hh in range(2):
                                    l = 2 * p + hh
                                    vap = r(
                                        v_sb[
                                            :,
                                            j * VW + l * (DK + 1) : j * VW
                                            + (l + 1) * (DK + 1),
                                        ]
                                    )
                                    if rdiag < 0:
                                        nc.tensor.matmul(
                                            pz[hh][:],
                                            vap,
                                            r(ets[hh][:]),
                                            start=(j == 0),
                                            stop=False,
                                            skip_group_check=True,
                                        )
                                    else:
                                        nc.tensor.matmul(
                                            pz[hh][:, rdiag * SK : (rdiag + 1) * SK],
                                            vap,
                                            r(ets[hh][:, rdiag * SK : (rdiag + 1) * SK]),
                                            start=(j == 0),
                                            stop=True,
                                            skip_group_check=True,
                                        )
                                        if rdiag < 3:
                                            nc.tensor.matmul(
                                                pz[hh][:, (rdiag + 1) * SK : TQ],
                                                vap,
                                                r(ets[hh][:, (rdiag + 1) * SK : TQ]),
                                                start=(j == 0),
                                                stop=False,
                                                skip_group_check=True,
                                            )
                            for hh in range(2):
                                # normalize: zT = zT_unnorm * (1/denom)
                                rcp = rcp_pool.tile(
                                    [1, TQ], mmdt, tag="rcp", name="rcp"
                                )
                                with nc.allow_low_precision(reason="fp32r recip"):
                                    nc.vector.reciprocal(
                                        rcp[:], pz[hh][DK : DK + 1, :]
                                    )
                                rb_sb = rbs_pool.tile(
                                    [DK, TQ], f32, tag="rbs", name="rbs"
                                )
                                nc.gpsimd.partition_broadcast(
                                    rb_sb[:], rcp.bitcast(f32)[:]
                                )
                                nc.vector.tensor_mul(
                                    zT_sb[p][hh * DK : (hh + 1) * DK, sl],
                                    pz[hh][0:DK, :],
                                    rb_sb[:],
                                )

                        # ---- O projection, pipelined one tq chunk behind ----
                        if tcq >= 1:
                            _o_proj(tcq - 1)
                    _o_proj(TC - 1)

            if reps == 1:
                _phases()
            else:
                with tc.For_i(0, reps, 1):
                    _phases()

    nc.compile()
    return nc


def _get_program(t=T, d=D, reps=1):
    key = (t, d, USE_FP32R, MASK_ENGINE, RB_MODE, reps)
    if key not in _PROGRAM_CACHE:
        _PROGRAM_CACHE[key] = _build_program(t, d, reps)
    return _PROGRAM_CACHE[key]


def _make_masks():
    # lower-triangular keep-mask for the exact diagonal 128x128 block
    i = np.arange(SK)[:, None]
    j = np.arange(SK)[None, :]
    return (i <= j).astype(np.float32)  # [128, 128]


def _core_inputs(x, Qs, Qbs, Ks, Kbs, Vs, Vbs, O, b, g, mask_host):
    hs = slice(H * g, H * (g + 1))
    xT_b = np.ascontiguousarray(x[b].T)  # [D, T]
    wq_g = np.ascontiguousarray(Qs[hs].transpose(1, 0, 2).reshape(D, H * DK))
    wk_g = np.ascontiguousarray(Ks[hs].transpose(1, 0, 2).reshape(D, H * DK))
    wv_g = np.ascontiguousarray(Vs[hs].transpose(1, 0, 2).reshape(D, H * DK))
    wo_g = np.ascontiguousarray(O[hs].reshape(H * DK, D))
    qb_flat = Qbs[hs].reshape(H * DK)
    kb_flat = Kbs[hs].reshape(H * DK)
    qb_g = np.ascontiguousarray(np.stack([qb_flat[0:128], qb_flat[128:256]], axis=1))
    kb_g = np.ascontiguousarray(np.stack([kb_flat[0:128], kb_flat[128:256]], axis=1))
    vb_g = np.ascontiguousarray(Vbs[hs].reshape(1, H * DK))
    return {
        "xT": xT_b,
        "wq": wq_g,
        "wk": wk_g,
        "wv": wv_g,
        "wo": wo_g,
        "qb": qb_g,
        "kb": kb_g,
        "vb": vb_g,
        "mask": mask_host,
    }


def _build_in_maps(x, Qs, Qbs, Ks, Kbs, Vs, Vbs, O_):
    mask_host = _make_masks()
    in_maps = []
    for core in range(NCORES):
        b, g = divmod(core, NH // H)
        in_maps.append(_core_inputs(x, Qs, Qbs, Ks, Kbs, Vs, Vbs, O_, b, g, mask_host))
    return in_maps


def benchmark(inputs, iters=20, warmup=3, reps=1):
    """Time repeated on-device executions with device-resident inputs.

    Returns (best_per_call_s, avg_per_call_s, burst_amortized_s).
    """
    import time

    import jax
    import numpy as _np
    from jax.experimental.shard_map import shard_map
    from jax.sharding import Mesh, PartitionSpec

    from concourse import bass2jax, mybir

    nc = _get_program(reps=reps)
    x = np.asarray(inputs["normalized_resid_pre"], np.float32)
    in_maps = _build_in_maps(
        x,
        np.asarray(inputs["Qs"], np.float32),
        np.asarray(inputs["Qbs"], np.float32),
        np.asarray(inputs["Ks"], np.float32),
        np.asarray(inputs["Kbs"], np.float32),
        np.asarray(inputs["Vs"], np.float32),
        np.asarray(inputs["Vbs"], np.float32),
        np.asarray(inputs["O"], np.float32),
    )

    bass2jax.install_neuronx_cc_hook()
    partition_name = nc.partition_id_tensor.name if nc.partition_id_tensor else None
    in_names, out_names, out_avals, zero_outs = [], [], [], []
    for alloc in nc.m.functions[0].allocations:
        if not isinstance(alloc, mybir.MemoryLocationSet):
            continue
        name = alloc.memorylocations[0].name
        if alloc.kind == "ExternalInput":
            if name != partition_name:
                in_names.append(name)
        elif alloc.kind == "ExternalOutput":
            out_names.append(name)
            dt = mybir.dt.np(alloc.dtype)
            out_avals.append(jax.core.ShapedArray(tuple(alloc.tensor_shape), dt))
            zero_outs.append(_np.zeros(tuple(alloc.tensor_shape), dt))
    n_params = len(in_names)
    all_names = in_names + out_names
    if partition_name is not None:
        all_names = all_names + [partition_name]

    def _body(*args):
        operands = list(args)
        if partition_name is not None:
            operands.append(bass2jax.partition_id_tensor())
        outs = bass2jax._bass_exec_p.bind(
            *operands,
            out_avals=tuple(out_avals),
            in_names=tuple(all_names),
            out_names=tuple(out_names),
            lowering_input_output_aliases=(),
            sim_require_finite=True,
            sim_require_nnan=True,
            nc=nc,
        )
        return tuple(outs)

    devices = jax.devices()[:NCORES]
    mesh = Mesh(_np.asarray(devices), ("core",))
    n_all = n_params + len(out_names)
    sharded = jax.jit(
        shard_map(
            _body,
            mesh=mesh,
            in_specs=(PartitionSpec("core"),) * n_all,
            out_specs=(PartitionSpec("core"),) * len(out_names),
            check_rep=False,
        ),
        keep_unused=True,
    )
    concat_in = [
        _np.concatenate([_np.asarray(in_maps[c][nm]) for c in range(NCORES)], axis=0)
        for nm in in_names
    ]
    sharding = jax.sharding.NamedSharding(mesh, PartitionSpec("core"))
    dev_in = [jax.device_put(a, sharding) for a in concat_in]
    dev_zeros = [
        jax.device_put(
            _np.zeros((NCORES * z.shape[0], *z.shape[1:]), z.dtype), sharding
        )
        for z in zero_outs
    ]

    for _ in range(warmup):
        jax.block_until_ready(sharded(*dev_in, *dev_zeros))
    times = []
    for _ in range(iters):
        t0 = time.perf_counter()
        jax.block_until_ready(sharded(*dev_in, *dev_zeros))
        times.append(time.perf_counter() - t0)
    t0 = time.perf_counter()
    rs = [sharded(*dev_in, *dev_zeros) for _ in range(iters)]
    jax.block_until_ready(rs)
    burst = (time.perf_counter() - t0) / iters
    return min(times), sum(times) / len(times), burst


def kernel(normalized_resid_pre, Qs, Qbs, Ks, Kbs, Vs, Vbs, O, Ob):
    from concourse.bass_utils import run_bass_kernel_spmd

    x = np.asarray(normalized_resid_pre, dtype=np.float32)
    Qs, Qbs = np.asarray(Qs, np.float32), np.asarray(Qbs, np.float32)
    Ks, Kbs = np.asarray(Ks, np.float32), np.asarray(Kbs, np.float32)
    Vs, Vbs = np.asarray(Vs, np.float32), np.asarray(Vbs, np.float32)
    O_, Ob = np.asarray(O, np.float32), np.asarray(Ob, np.float32)

    nc = _get_program()
    in_maps = _build_in_maps(x, Qs, Qbs, Ks, Kbs, Vs, Vbs, O_)

    res = run_bass_kernel_spmd(
        nc, in_maps, core_ids=list(range(NCORES)), trace=TRACE, **TRACE_KWARGS
    )
    kernel.last_results = res

    out = np.zeros((B, T, D), dtype=np.float32)
    for core in range(NCORES):
        b, g = divmod(core, NH // H)
        out[b] += res.results[core]["out"]
    out += Ob[None, None, :]
    return out



# revision 4
# speedup vs baseline: 1.6909x; 1.6909x over previous
"""Decoder-only attention kernel for Trainium2 (8 NeuronCores).

Sharding: tensor-parallel over heads (4 groups of 4 heads) x data-parallel
over batch (2), = 8 cores. Each core computes, for its batch b and its 4
heads, the partial output  sum_h (softmax(causal(q_h k_h^T / 8)) v_h) @ O_h
as a [T, D] array; the host sums the 4 partials per batch and adds Ob.

v2 design notes (vs the fp32r baseline at ~240us):
  - All matmul operands are bf16 (PSUM accumulation stays fp32). On real
    TRN2 an fp32r moving operand streams at ~2 cycles/column; bf16 streams
    at 1 — this alone halves PE time. Tolerance is 2e-2; bf16 lands ~1e-3.
  - Single software-pipelined instruction stream: the QKV projection
    matmuls of chunk tcq+1 and the O-projection matmuls of chunk tcq-1 are
    issued as *filler* between attention j-steps. The PE engine executes
    in order, so filler keeps it busy while ACT computes exp() — and a
    continuously-busy PE keeps the HAM clock gate at K=8/8 (2.4 GHz). The
    fp32r baseline ran the attention phase almost entirely at K=4/8.
  - Scores for a head-pair land in one [128, 1024] PSUM tile (two banks),
    so one ACT instruction exponentiates both heads (amortizes the ~352
    cycle ACT fixed overhead). Diagonal j-blocks are issued width-trimmed.
  - Softmax denominator comes free as a ones-column appended to v in the
    z matmul; 1/denom uses reciprocal_approx_fast (~5x cheaper on DVE).
  - causal: upper-triangle (s,tq) blocks skipped; diagonal 128x128 blocks
    multiplied post-exp by a host-precomputed 0/1 mask.
"""

import sys

import numpy as np

if "/opt/trn_rl_repo" not in sys.path:
    sys.path.insert(0, "/opt/trn_rl_repo")

# Model shapes (hardcoded; kernel.py must be self-contained).
B = 2
T = 2048
D = 1024
NH = 16  # total heads
H = 4  # heads per core
DK = 64
NCORES = 8

TQ = 512  # tq chunk (matmul moving free dim)
SK = 128  # s chunk (partition dim of scoresT)

_PROGRAM_CACHE = {}

# test.py can flip these before calling kernel()
TRACE = False
TRACE_KWARGS = {}
FILLERS_PER_STEP = 2


def _build_program(t=T, d=D, reps=1):
    import concourse.bass as bass  # noqa: F401
    import concourse.mybir as mybir
    import concourse.tile as tile
    from concourse import bacc

    f32 = mybir.dt.float32
    bf16 = mybir.dt.bfloat16

    P = 128
    DC = d // P  # d_model chunks (contraction for projections)
    SC = t // SK  # s chunks
    TC = t // TQ  # tq chunks
    VW = H * (DK + 1)  # v block width per s-chunk: 4 heads x (64 v + 1 ones)
    m = H * DK

    nc = bacc.Bacc(
        "TRN2",
        target_bir_lowering=False,
        debug=False,
        enable_asserts=True,
        num_devices=NCORES,
    )

    xT = nc.dram_tensor("xT", [d, t], bf16, kind="ExternalInput").ap()
    wq = nc.dram_tensor("wq", [d, m], bf16, kind="ExternalInput").ap()
    wk = nc.dram_tensor("wk", [d, m], bf16, kind="ExternalInput").ap()
    wv = nc.dram_tensor("wv", [d, m], bf16, kind="ExternalInput").ap()
    wo = nc.dram_tensor("wo", [m, d], bf16, kind="ExternalInput").ap()
    qb = nc.dram_tensor("qb", [P, 2], f32, kind="ExternalInput").ap()
    kb = nc.dram_tensor("kb", [P, 2], f32, kind="ExternalInput").ap()
    vb = nc.dram_tensor("vb", [1, m], bf16, kind="ExternalInput").ap()
    mask = nc.dram_tensor("mask", [P, P], bf16, kind="ExternalInput").ap()
    out = nc.dram_tensor("out", [t, d], f32, kind="ExternalOutput").ap()

    with tile.TileContext(nc) as tc:
        from contextlib import ExitStack

        ctx = ExitStack()
        with ctx:
            const = ctx.enter_context(tc.tile_pool(name="const", bufs=1))

            # ---- resident SBUF tensors ----
            xT_sb = [
                const.tile([P, t], bf16, tag=f"xT{c}", name=f"xT{c}")
                for c in range(DC)
            ]
            wq_sb = const.tile([P, DC * m], bf16, tag="wq")
            wk_sb = const.tile([P, DC * m], bf16, tag="wk")
            wv_sb = const.tile([P, DC * m], bf16, tag="wv")
            wo_sb = const.tile([P, 2 * d], bf16, tag="wo")
            qb_sb = const.tile([P, 2], f32, tag="qb")
            kb_sb = const.tile([P, 2], f32, tag="kb")
            vb_sb = const.tile([1, m], bf16, tag="vb")
            mask_sb = const.tile([P, P], bf16, tag="mask")
            ones_row = const.tile([1, P], bf16, tag="ones_row")
            v_sb = const.tile([P, SC * VW], bf16, tag="v")
            qT_sb = [
                const.tile([P, t], bf16, tag=f"qT{p}", name=f"qT{p}") for p in range(2)
            ]
            kT_sb = [
                const.tile([P, t], bf16, tag=f"kT{p}", name=f"kT{p}") for p in range(2)
            ]
            zT_sb = [
                const.tile([P, t], bf16, tag=f"zT{p}", name=f"zT{p}") for p in range(2)
            ]

            # ---- input DMAs: weights/constants first, then xT streamed in
            # tp-major [128, TQ] pieces so projections start early
            nc.sync.dma_start(vb_sb[:], vb[:])
            for c in range(DC):
                nc.sync.dma_start(
                    wv_sb[:, c * m : (c + 1) * m], wv[c * P : (c + 1) * P, :]
                )
            for c in range(DC):
                nc.sync.dma_start(
                    xT_sb[c][:, 0:TQ], xT[c * P : (c + 1) * P, 0:TQ]
                )
            for c in range(DC):
                nc.sync.dma_start(
                    wq_sb[:, c * m : (c + 1) * m], wq[c * P : (c + 1) * P, :]
                )
                nc.sync.dma_start(
                    wk_sb[:, c * m : (c + 1) * m], wk[c * P : (c + 1) * P, :]
                )
            nc.sync.dma_start(qb_sb[:], qb[:])
            nc.sync.dma_start(kb_sb[:], kb[:])
            nc.sync.dma_start(mask_sb[:], mask[:])
            if TC > 1:
                for c in range(DC):
                    nc.sync.dma_start(
                        xT_sb[c][:, TQ : 2 * TQ],
                        xT[c * P : (c + 1) * P, TQ : 2 * TQ],
                    )
            for kc in range(2):
                nc.sync.dma_start(
                    wo_sb[:, kc * d : (kc + 1) * d], wo[kc * P : (kc + 1) * P, :]
                )
            for tp in range(2, TC):
                for c in range(DC):
                    nc.sync.dma_start(
                        xT_sb[c][:, tp * TQ : (tp + 1) * TQ],
                        xT[c * P : (c + 1) * P, tp * TQ : (tp + 1) * TQ],
                    )
            ones_stage = const.tile([P, 1], f32, tag="ones_stage")
            nc.vector.memset(ones_stage[:], 1.0)
            nc.vector.tensor_copy(ones_row[:], ones_stage[0:1, :].to_broadcast([1, P]))
            # ones columns of the v blocks (col 64 of each head's 65-wide slot)
            nc.vector.tensor_copy(
                v_sb.rearrange("p (j h c) -> p j h c", h=H, c=DK + 1)[:, :, :, DK :],
                ones_stage[:, None, None, :].to_broadcast([P, SC, H, 1]),
            )

            with (
                tc.tile_pool(name="fp", bufs=2, space="PSUM") as fpool,
                tc.tile_pool(name="ps", bufs=1, space="PSUM") as ps_pool,
                tc.tile_pool(name="pz", bufs=4, space="PSUM") as pz_pool,
                tc.tile_pool(name="expt", bufs=4) as exp_pool,
                tc.tile_pool(name="rcp", bufs=4) as rcp_pool,
                tc.tile_pool(name="rbs", bufs=4) as rbs_pool,
                tc.tile_pool(name="osb", bufs=6) as out_pool,
            ):

                def proj_units(tp):
                    """QKV projection for tq chunk `tp`; yields once per PE MM."""
                    sl = slice(tp * TQ, (tp + 1) * TQ)
                    # v projection (+v bias folded in as one K=1 matmul)
                    for i in range(4 * tp, 4 * tp + 4):
                        pv = fpool.tile([P, m], f32, tag="f", name="pv")
                        for c in range(DC):
                            nc.tensor.matmul(
                                pv[:],
                                xT_sb[c][:, i * P : (i + 1) * P],
                                wv_sb[:, c * m : (c + 1) * m],
                                start=(c == 0),
                                stop=False,
                            )
                            yield
                        nc.tensor.matmul(
                            pv[:],
                            ones_row[0:1, :],
                            vb_sb[0:1, :],
                            start=False,
                            stop=True,
                        )
                        nc.vector.tensor_copy(
                            v_sb.rearrange("p (j h c) -> p j h c", h=H, c=DK + 1)[
                                :, i, :, 0:DK
                            ],
                            pv.rearrange("p (h c) -> p h c", c=DK),
                        )
                        yield
                    # q/k projections
                    for p in range(2):
                        pq = fpool.tile([P, TQ], f32, tag="f", name="pq")
                        for c in range(DC):
                            nc.tensor.matmul(
                                pq[:],
                                wq_sb[:, c * m + p * P : c * m + (p + 1) * P],
                                xT_sb[c][:, sl],
                                start=(c == 0),
                                stop=(c == DC - 1),
                            )
                            yield
                        nc.vector.tensor_scalar_add(
                            qT_sb[p][:, sl], pq[:], qb_sb[:, p : p + 1]
                        )
                        pk = fpool.tile([P, TQ], f32, tag="f", name="pk")
                        for c in range(DC):
                            nc.tensor.matmul(
                                pk[:],
                                wk_sb[:, c * m + p * P : c * m + (p + 1) * P],
                                xT_sb[c][:, sl],
                                start=(c == 0),
                                stop=(c == DC - 1),
                            )
                            yield
                        nc.vector.tensor_scalar_add(
                            kT_sb[p][:, sl], pk[:], kb_sb[:, p : p + 1]
                        )

                def o_units(tq_c):
                    """O projection for tq chunk `tq_c`; yields once per PE MM."""
                    for i in range(4 * tq_c, 4 * tq_c + 4):
                        for d2 in range(d // TQ):
                            po = fpool.tile([P, TQ], f32, tag="f", name="po")
                            for kc in range(2):
                                nc.tensor.matmul(
                                    po[:],
                                    zT_sb[kc][:, i * P : (i + 1) * P],
                                    wo_sb[
                                        :,
                                        kc * d + d2 * TQ : kc * d + (d2 + 1) * TQ,
                                    ],
                                    start=(kc == 0),
                                    stop=(kc == 1),
                                )
                                yield
                            ot = out_pool.tile([P, TQ], f32, tag="osb", name="ot")
                            nc.vector.tensor_copy(ot[:], po[:])
                            nc.sync.dma_start(
                                out[i * P : (i + 1) * P, d2 * TQ : (d2 + 1) * TQ],
                                ot[:],
                            )

                def _exhaust(gen):
                    for _ in gen:
                        pass

                def _pull(gens, n):
                    """Issue up to n filler matmuls from the generator list."""
                    done = 0
                    while done < n and gens:
                        try:
                            next(gens[0])
                            done += 1
                        except StopIteration:
                            gens.pop(0)

                # ---- prologue: project chunk 0 (ACT is idle here anyway) ----
                _exhaust(proj_units(0))

                o_carry = []  # deferred O-projection units (safe to roll over)
                for tcq in range(TC):
                    sl = slice(tcq * TQ, (tcq + 1) * TQ)
                    nsc = min(SC, 4 * tcq + 4)
                    # filler for this window: O(tcq-1) first (rolls over),
                    # then proj(tcq+1) (must complete inside this window)
                    if tcq >= 1:
                        o_carry.append(o_units(tcq - 1))
                    proj_gen = [proj_units(tcq + 1)] if tcq + 1 < TC else []

                    for p in range(2):
                        pz = [
                            pz_pool.tile([DK + 1, TQ], f32, tag="pz", name="pz")
                            for _ in range(2)
                        ]
                        prev = None  # (j, et, rdiag)

                        def _z_mms(j, et, rdiag, pz=pz, tcq=tcq, p=p):
                            for hh in range(2):
                                l = 2 * p + hh
                                vap = v_sb[
                                    :,
                                    j * VW + l * (DK + 1) : j * VW
                                    + (l + 1) * (DK + 1),
                                ]
                                if rdiag < 0:
                                    nc.tensor.matmul(
                                        pz[hh][:],
                                        vap,
                                        et[:, hh * TQ : (hh + 1) * TQ],
                                        start=(j == 0),
                                        stop=False,
                                        skip_group_check=True,
                                    )
                                else:
                                    nc.tensor.matmul(
                                        pz[hh][:, rdiag * SK : (rdiag + 1) * SK],
                                        vap,
                                        et[
                                            :,
                                            hh * TQ
                                            + rdiag * SK : hh * TQ
                                            + (rdiag + 1) * SK,
                                        ],
                                        start=(j == 0),
                                        stop=True,
                                        skip_group_check=True,
                                    )
                                    if rdiag < 3:
                                        nc.tensor.matmul(
                                            pz[hh][:, (rdiag + 1) * SK : TQ],
                                            vap,
                                            et[
                                                :,
                                                hh * TQ
                                                + (rdiag + 1) * SK : hh * TQ
                                                + TQ,
                                            ],
                                            start=(j == 0),
                                            stop=False,
                                            skip_group_check=True,
                                        )

                        for j in range(nsc):
                            rdiag = j - 4 * tcq
                            c0 = max(rdiag, 0) * SK
                            ps = ps_pool.tile([P, 2 * TQ], f32, tag="ps", name="ps")
                            # two heads' K=64 scores matmuls sit at PE row
                            # groups 0-63 / 64-127: issue back-to-back
                            for hh in range(2):
                                nc.tensor.matmul(
                                    ps[:, hh * TQ + c0 : (hh + 1) * TQ],
                                    kT_sb[p][
                                        hh * DK : (hh + 1) * DK,
                                        j * SK : (j + 1) * SK,
                                    ],
                                    qT_sb[p][
                                        hh * DK : (hh + 1) * DK,
                                        tcq * TQ + c0 : (tcq + 1) * TQ,
                                    ],
                                    start=True,
                                    stop=True,
                                )
                            et = exp_pool.tile([P, 2 * TQ], bf16, tag="et", name="et")
                            if c0 == 0:
                                nc.scalar.activation(
                                    et[:],
                                    ps[:],
                                    mybir.ActivationFunctionType.Exp,
                                    scale=0.125,
                                )
                            else:
                                for hh in range(2):
                                    nc.scalar.activation(
                                        et[:, hh * TQ + c0 : (hh + 1) * TQ],
                                        ps[:, hh * TQ + c0 : (hh + 1) * TQ],
                                        mybir.ActivationFunctionType.Exp,
                                        scale=0.125,
                                    )
                            if rdiag >= 0:
                                # mask only the 128x128 diagonal block
                                for hh in range(2):
                                    nc.vector.tensor_tensor(
                                        et[
                                            :,
                                            hh * TQ
                                            + rdiag * SK : hh * TQ
                                            + (rdiag + 1) * SK,
                                        ],
                                        et[
                                            :,
                                            hh * TQ
                                            + rdiag * SK : hh * TQ
                                            + (rdiag + 1) * SK,
                                        ],
                                        mask_sb[:],
                                        op=mybir.AluOpType.mult,
                                    )
                            # filler matmuls keep the PE busy while ACT exps
                            _pull(proj_gen or o_carry, FILLERS_PER_STEP)
                            if prev is not None:
                                _z_mms(*prev)
                            prev = (j, et, rdiag)
                        _z_mms(*prev)

                        # normalize: zT = zT_unnorm * (1/denom)
                        for hh in range(2):
                            dn = rcp_pool.tile([1, TQ], f32, tag="dn", name="dn")
                            nc.vector.tensor_copy(dn[:], pz[hh][DK : DK + 1, :])
                            rcp = rcp_pool.tile([1, TQ], f32, tag="rcp", name="rcp")
                            # custom-DVE op: SBUF->SBUF only (PSUM src
                            # diverges on HW)
                            nc.vector.reciprocal_approx_fast(rcp[:], dn[:])
                            rb_sb = rbs_pool.tile([DK, TQ], f32, tag="rbs", name="rbs")
                            nc.gpsimd.partition_broadcast(rb_sb[:], rcp[:])
                            with nc.allow_low_precision(reason="bf16 z"):
                                nc.vector.tensor_mul(
                                    zT_sb[p][hh * DK : (hh + 1) * DK, sl],
                                    pz[hh][0:DK, :],
                                    rb_sb[:],
                                )

                    # proj(tcq+1) must be fully issued before attn(tcq+1)
                    for g in proj_gen:
                        _exhaust(g)

                for g in o_carry:
                    _exhaust(g)
                _exhaust(o_units(TC - 1))

    nc.compile()
    return nc


def _get_program(t=T, d=D, reps=1):
    key = (t, d, FILLERS_PER_STEP, reps)
    if key not in _PROGRAM_CACHE:
        _PROGRAM_CACHE[key] = _build_program(t, d, reps)
    return _PROGRAM_CACHE[key]


def _bf16(a):
    import ml_dtypes

    return np.ascontiguousarray(a).astype(ml_dtypes.bfloat16)


def _make_masks():
    # lower-triangular keep-mask for the exact diagonal 128x128 block
    i = np.arange(SK)[:, None]
    j = np.arange(SK)[None, :]
    return (i <= j).astype(np.float32)  # [128, 128]


def _core_inputs(x, Qs, Qbs, Ks, Kbs, Vs, Vbs, O, b, g, mask_host):
    hs = slice(H * g, H * (g + 1))
    xT_b = np.ascontiguousarray(x[b].T)  # [D, T]
    wq_g = np.ascontiguousarray(Qs[hs].transpose(1, 0, 2).reshape(D, H * DK))
    wk_g = np.ascontiguousarray(Ks[hs].transpose(1, 0, 2).reshape(D, H * DK))
    wv_g = np.ascontiguousarray(Vs[hs].transpose(1, 0, 2).reshape(D, H * DK))
    wo_g = np.ascontiguousarray(O[hs].reshape(H * DK, D))
    qb_flat = Qbs[hs].reshape(H * DK)
    kb_flat = Kbs[hs].reshape(H * DK)
    qb_g = np.ascontiguousarray(np.stack([qb_flat[0:128], qb_flat[128:256]], axis=1))
    kb_g = np.ascontiguousarray(np.stack([kb_flat[0:128], kb_flat[128:256]], axis=1))
    vb_g = np.ascontiguousarray(Vbs[hs].reshape(1, H * DK))
    return {
        "xT": _bf16(xT_b),
        "wq": _bf16(wq_g),
        "wk": _bf16(wk_g),
        "wv": _bf16(wv_g),
        "wo": _bf16(wo_g),
        "qb": qb_g.astype(np.float32),
        "kb": kb_g.astype(np.float32),
        "vb": _bf16(vb_g),
        "mask": _bf16(mask_host),
    }


def _build_in_maps(x, Qs, Qbs, Ks, Kbs, Vs, Vbs, O_):
    mask_host = _make_masks()
    in_maps = []
    for core in range(NCORES):
        b, g = divmod(core, NH // H)
        in_maps.append(_core_inputs(x, Qs, Qbs, Ks, Kbs, Vs, Vbs, O_, b, g, mask_host))
    return in_maps


def kernel(normalized_resid_pre, Qs, Qbs, Ks, Kbs, Vs, Vbs, O, Ob):
    from concourse.bass_utils import run_bass_kernel_spmd

    x = np.asarray(normalized_resid_pre, dtype=np.float32)
    Qs, Qbs = np.asarray(Qs, np.float32), np.asarray(Qbs, np.float32)
    Ks, Kbs = np.asarray(Ks, np.float32), np.asarray(Kbs, np.float32)
    Vs, Vbs = np.asarray(Vs, np.float32), np.asarray(Vbs, np.float32)
    O_, Ob = np.asarray(O, np.float32), np.asarray(Ob, np.float32)

    nc = _get_program()
    in_maps = _build_in_maps(x, Qs, Qbs, Ks, Kbs, Vs, Vbs, O_)

    res = run_bass_kernel_spmd(
        nc, in_maps, core_ids=list(range(NCORES)), trace=TRACE, **TRACE_KWARGS
    )
    kernel.last_results = res

    out = np.zeros((B, T, D), dtype=np.float32)
    for core in range(NCORES):
        b, g = divmod(core, NH // H)
        out[b] += res.results[core]["out"]
    out += Ob[None, None, :]
    return out


# revision 5
# speedup vs baseline: 1.7744x; 1.0494x over previous
"""Decoder-only attention kernel for Trainium2 (8 NeuronCores).

Sharding: tensor-parallel over heads (4 groups of 4 heads) x data-parallel
over batch (2), = 8 cores. Each core computes, for its batch b and its 4
heads, the partial output  sum_h (softmax(causal(q_h k_h^T / 8)) v_h) @ O_h
as a [T, D] array; the host sums the 4 partials per batch and adds Ob.

v3 design notes (vs the fp32r baseline at ~240us):
  - All matmul operands are bf16 (PSUM accumulation stays fp32). On real
    TRN2 an fp32r moving operand streams at ~2 cycles/column; bf16 streams
    at 1 — this alone halves PE time. Tolerance is 2e-2; bf16 lands ~4e-3.
  - Single software-pipelined instruction stream: the QKV projection
    matmuls of chunk tcq+1 are spread evenly across attention j-steps of
    chunk tcq, and O-projection matmuls are deferred into the last
    (otherwise filler-starved) window. The PE executes in order, so filler
    keeps it busy while ACT computes exp() — and a continuously-busy PE
    keeps the HAM clock gate at K=8/8 (2.4 GHz).
  - Scores for a head-pair land in one [128, 1024] PSUM tile (two banks),
    so one ACT instruction exponentiates both heads (amortizes the ~352
    cycle ACT fixed overhead). Diagonal j-blocks are issued width-trimmed.
  - Softmax denominator comes free as a ones-column appended to v in the
    z matmul; 1/denom uses reciprocal_approx_fast (~5x cheaper on DVE;
    must run SBUF->SBUF — PSUM-sourced custom-DVE ops diverge on HW).
  - Input tensors stream as one strided DMA descriptor each (per-DMA
    overhead dominates small transfers), split across the two HWDGE
    queues (sync + scalar) so weights and xT arrive in parallel.
  - causal: upper-triangle (s,tq) blocks skipped; diagonal 128x128 blocks
    multiplied post-exp by a host-precomputed 0/1 mask.
"""

import sys

import numpy as np

if "/opt/trn_rl_repo" not in sys.path:
    sys.path.insert(0, "/opt/trn_rl_repo")

# Model shapes (hardcoded; kernel.py must be self-contained).
B = 2
T = 2048
D = 1024
NH = 16  # total heads
H = 4  # heads per core
DK = 64
NCORES = 8

TQ = 512  # tq chunk (matmul moving free dim)
SK = 128  # s chunk (partition dim of scoresT)

_PROGRAM_CACHE = {}

# test.py can flip these before calling kernel()
TRACE = False
TRACE_KWARGS = {}


def _build_program(t=T, d=D, reps=1):
    import concourse.bass as bass  # noqa: F401
    import concourse.mybir as mybir
    import concourse.tile as tile
    from concourse import bacc

    f32 = mybir.dt.float32
    bf16 = mybir.dt.bfloat16

    P = 128
    DC = d // P  # d_model chunks (contraction for projections)
    SC = t // SK  # s chunks
    TC = t // TQ  # tq chunks
    VW = H * (DK + 1)  # v block width per s-chunk: 4 heads x (64 v + 1 ones)
    m = H * DK

    nc = bacc.Bacc(
        "TRN2",
        target_bir_lowering=False,
        debug=False,
        enable_asserts=True,
        num_devices=NCORES,
    )

    xT = nc.dram_tensor("xT", [d, t], bf16, kind="ExternalInput").ap()
    wq = nc.dram_tensor("wq", [d, m], bf16, kind="ExternalInput").ap()
    wk = nc.dram_tensor("wk", [d, m], bf16, kind="ExternalInput").ap()
    wv = nc.dram_tensor("wv", [d, m], bf16, kind="ExternalInput").ap()
    wo = nc.dram_tensor("wo", [m, d], bf16, kind="ExternalInput").ap()
    qb = nc.dram_tensor("qb", [P, 2], f32, kind="ExternalInput").ap()
    kb = nc.dram_tensor("kb", [P, 2], f32, kind="ExternalInput").ap()
    vb = nc.dram_tensor("vb", [1, m], bf16, kind="ExternalInput").ap()
    mask = nc.dram_tensor("mask", [P, P], bf16, kind="ExternalInput").ap()
    out = nc.dram_tensor("out", [t, d], f32, kind="ExternalOutput").ap()

    with tile.TileContext(nc) as tc:
        from contextlib import ExitStack

        ctx = ExitStack()
        with ctx:
            const = ctx.enter_context(tc.tile_pool(name="const", bufs=1))

            # ---- resident SBUF tensors ----
            xT_sb = const.tile([P, DC * t], bf16, tag="xT")  # [p, c, t]
            wq_sb = const.tile([P, DC * m], bf16, tag="wq")
            wk_sb = const.tile([P, DC * m], bf16, tag="wk")
            wv_sb = const.tile([P, DC * m], bf16, tag="wv")
            wo_sb = const.tile([P, 2 * d], bf16, tag="wo")
            qb_sb = const.tile([P, 2], f32, tag="qb")
            kb_sb = const.tile([P, 2], f32, tag="kb")
            vb_sb = const.tile([1, m], bf16, tag="vb")
            mask_sb = const.tile([P, P], bf16, tag="mask")
            ones_row = const.tile([1, P], bf16, tag="ones_row")
            v_sb = const.tile([P, SC * VW], bf16, tag="v")
            qT_sb = [
                const.tile([P, t], bf16, tag=f"qT{p}", name=f"qT{p}") for p in range(2)
            ]
            kT_sb = [
                const.tile([P, t], bf16, tag=f"kT{p}", name=f"kT{p}") for p in range(2)
            ]
            zT_sb = [
                const.tile([P, t], bf16, tag=f"zT{p}", name=f"zT{p}") for p in range(2)
            ]

            def xc(c):  # xT chunk c: [128, t] slice of the packed tile
                return xT_sb[:, c * t : (c + 1) * t]

            # ---- input DMAs: one strided descriptor per tensor, split
            # across the two HWDGE queues (sync: xT; scalar: weights).
            # DRAM [d, n] with d = c*128 + p  ->  SBUF [p, c*n + j].
            def fold(dram_ap, n, parts):
                return dram_ap.rearrange("(c p) n -> p c n", p=parts)

            nc.scalar.dma_start(vb_sb[:], vb[:])
            nc.scalar.dma_start(
                wv_sb.rearrange("p (c n) -> p c n", c=DC), fold(wv, m, P)
            )
            # xT arrives tq-chunk-major so projections can chase the stream
            for tp in range(TC):
                sl = slice(tp * TQ, (tp + 1) * TQ)
                nc.sync.dma_start(
                    xT_sb.rearrange("p (c n) -> p c n", c=DC)[:, :, sl],
                    fold(xT, t, P)[:, :, sl],
                )
            nc.scalar.dma_start(
                wq_sb.rearrange("p (c n) -> p c n", c=DC), fold(wq, m, P)
            )
            nc.scalar.dma_start(
                wk_sb.rearrange("p (c n) -> p c n", c=DC), fold(wk, m, P)
            )
            nc.scalar.dma_start(qb_sb[:], qb[:])
            nc.scalar.dma_start(kb_sb[:], kb[:])
            nc.scalar.dma_start(mask_sb[:], mask[:])
            nc.scalar.dma_start(
                wo_sb.rearrange("p (c n) -> p c n", c=2), fold(wo, d, P)
            )
            ones_stage = const.tile([P, 1], f32, tag="ones_stage")
            nc.vector.memset(ones_stage[:], 1.0)
            nc.vector.tensor_copy(ones_row[:], ones_stage[0:1, :].to_broadcast([1, P]))
            # ones columns of the v blocks (col 64 of each head's 65-wide slot)
            nc.vector.tensor_copy(
                v_sb.rearrange("p (j h c) -> p j h c", h=H, c=DK + 1)[:, :, :, DK :],
                ones_stage[:, None, None, :].to_broadcast([P, SC, H, 1]),
            )

            with (
                tc.tile_pool(name="fp", bufs=2, space="PSUM") as fpool,
                tc.tile_pool(name="ps", bufs=1, space="PSUM") as ps_pool,
                tc.tile_pool(name="pz", bufs=4, space="PSUM") as pz_pool,
                tc.tile_pool(name="expt", bufs=4) as exp_pool,
                tc.tile_pool(name="rcp", bufs=4) as rcp_pool,
                tc.tile_pool(name="rbs", bufs=4) as rbs_pool,
                tc.tile_pool(name="osb", bufs=6) as out_pool,
            ):

                def proj_units(tp):
                    """QKV projection for tq chunk `tp`; yields once per PE MM."""
                    sl = slice(tp * TQ, (tp + 1) * TQ)
                    # v projection (+v bias folded in as one K=1 matmul)
                    for i in range(4 * tp, 4 * tp + 4):
                        pv = fpool.tile([P, m], f32, tag="f", name="pv")
                        for c in range(DC):
                            nc.tensor.matmul(
                                pv[:],
                                xc(c)[:, i * P : (i + 1) * P],
                                wv_sb[:, c * m : (c + 1) * m],
                                start=(c == 0),
                                stop=False,
                            )
                            yield
                        nc.tensor.matmul(
                            pv[:],
                            ones_row[0:1, :],
                            vb_sb[0:1, :],
                            start=False,
                            stop=True,
                        )
                        nc.vector.tensor_copy(
                            v_sb.rearrange("p (j h c) -> p j h c", h=H, c=DK + 1)[
                                :, i, :, 0:DK
                            ],
                            pv.rearrange("p (h c) -> p h c", c=DK),
                        )
                        yield
                    # q/k projections
                    for p in range(2):
                        pq = fpool.tile([P, TQ], f32, tag="f", name="pq")
                        for c in range(DC):
                            nc.tensor.matmul(
                                pq[:],
                                wq_sb[:, c * m + p * P : c * m + (p + 1) * P],
                                xc(c)[:, sl],
                                start=(c == 0),
                                stop=(c == DC - 1),
                            )
                            yield
                        nc.vector.tensor_scalar_add(
                            qT_sb[p][:, sl], pq[:], qb_sb[:, p : p + 1]
                        )
                        pk = fpool.tile([P, TQ], f32, tag="f", name="pk")
                        for c in range(DC):
                            nc.tensor.matmul(
                                pk[:],
                                wk_sb[:, c * m + p * P : c * m + (p + 1) * P],
                                xc(c)[:, sl],
                                start=(c == 0),
                                stop=(c == DC - 1),
                            )
                            yield
                        nc.vector.tensor_scalar_add(
                            kT_sb[p][:, sl], pk[:], kb_sb[:, p : p + 1]
                        )

                def o_units(tq_c, drain_engine="vector"):
                    """O projection for tq chunk `tq_c`; yields once per PE MM."""
                    for i in range(4 * tq_c, 4 * tq_c + 4):
                        for d2 in range(d // TQ):
                            po = fpool.tile([P, TQ], f32, tag="f", name="po")
                            for kc in range(2):
                                nc.tensor.matmul(
                                    po[:],
                                    zT_sb[kc][:, i * P : (i + 1) * P],
                                    wo_sb[
                                        :,
                                        kc * d + d2 * TQ : kc * d + (d2 + 1) * TQ,
                                    ],
                                    start=(kc == 0),
                                    stop=(kc == 1),
                                )
                                yield
                            ot = out_pool.tile([P, TQ], f32, tag="osb", name="ot")
                            if drain_engine == "scalar":
                                nc.scalar.copy(ot[:], po[:])
                            else:
                                nc.vector.tensor_copy(ot[:], po[:])
                            nc.sync.dma_start(
                                out[i * P : (i + 1) * P, d2 * TQ : (d2 + 1) * TQ],
                                ot[:],
                            )

                def _exhaust(gen):
                    for _ in gen:
                        pass

                def _pull(gens, n):
                    """Issue up to n filler matmuls; returns #issued."""
                    done = 0
                    while done < n and gens:
                        try:
                            next(gens[0])
                            done += 1
                        except StopIteration:
                            gens.pop(0)
                    return done

                # ---- prologue: project chunk 0 (ACT is idle here anyway) ----
                _exhaust(proj_units(0))

                # MM counts for even spreading of proj filler over a window
                PROJ_MMS = 4 * (DC + 1) + 2 * 2 * DC
                o_carry = []  # deferred O-projection units (roll over freely)
                for tcq in range(TC):
                    sl = slice(tcq * TQ, (tcq + 1) * TQ)
                    nsc = min(SC, 4 * tcq + 4)
                    if tcq >= 1:
                        o_carry.append(o_units(tcq - 1))
                    proj_gen = [proj_units(tcq + 1)] if tcq + 1 < TC else []
                    proj_left = PROJ_MMS if proj_gen else 0
                    steps_left = 2 * nsc

                    for p in range(2):
                        pz = [
                            pz_pool.tile([DK + 1, TQ], f32, tag="pz", name="pz")
                            for _ in range(2)
                        ]
                        prev = None  # (j, et, rdiag)

                        def _z_mms(j, et, rdiag, pz=pz, tcq=tcq, p=p):
                            for hh in range(2):
                                l = 2 * p + hh
                                vap = v_sb[
                                    :,
                                    j * VW + l * (DK + 1) : j * VW
                                    + (l + 1) * (DK + 1),
                                ]
                                if rdiag < 0:
                                    nc.tensor.matmul(
                                        pz[hh][:],
                                        vap,
                                        et[:, hh * TQ : (hh + 1) * TQ],
                                        start=(j == 0),
                                        stop=False,
                                        skip_group_check=True,
                                    )
                                else:
                                    nc.tensor.matmul(
                                        pz[hh][:, rdiag * SK : (rdiag + 1) * SK],
                                        vap,
                                        et[
                                            :,
                                            hh * TQ
                                            + rdiag * SK : hh * TQ
                                            + (rdiag + 1) * SK,
                                        ],
                                        start=(j == 0),
                                        stop=True,
                                        skip_group_check=True,
                                    )
                                    if rdiag < 3:
                                        nc.tensor.matmul(
                                            pz[hh][:, (rdiag + 1) * SK : TQ],
                                            vap,
                                            et[
                                                :,
                                                hh * TQ
                                                + (rdiag + 1) * SK : hh * TQ
                                                + TQ,
                                            ],
                                            start=(j == 0),
                                            stop=False,
                                            skip_group_check=True,
                                        )

                        for j in range(nsc):
                            rdiag = j - 4 * tcq
                            c0 = max(rdiag, 0) * SK
                            ps = ps_pool.tile([P, 2 * TQ], f32, tag="ps", name="ps")
                            # two heads' K=64 scores matmuls sit at PE row
                            # groups 0-63 / 64-127: issue back-to-back
                            for hh in range(2):
                                nc.tensor.matmul(
                                    ps[:, hh * TQ + c0 : (hh + 1) * TQ],
                                    kT_sb[p][
                                        hh * DK : (hh + 1) * DK,
                                        j * SK : (j + 1) * SK,
                                    ],
                                    qT_sb[p][
                                        hh * DK : (hh + 1) * DK,
                                        tcq * TQ + c0 : (tcq + 1) * TQ,
                                    ],
                                    start=True,
                                    stop=True,
                                )
                            et = exp_pool.tile([P, 2 * TQ], bf16, tag="et", name="et")
                            if c0 == 0:
                                nc.scalar.activation(
                                    et[:],
                                    ps[:],
                                    mybir.ActivationFunctionType.Exp,
                                    scale=0.125,
                                )
                            else:
                                for hh in range(2):
                                    nc.scalar.activation(
                                        et[:, hh * TQ + c0 : (hh + 1) * TQ],
                                        ps[:, hh * TQ + c0 : (hh + 1) * TQ],
                                        mybir.ActivationFunctionType.Exp,
                                        scale=0.125,
                                    )
                            if rdiag >= 0:
                                # mask only the 128x128 diagonal block
                                for hh in range(2):
                                    nc.vector.tensor_tensor(
                                        et[
                                            :,
                                            hh * TQ
                                            + rdiag * SK : hh * TQ
                                            + (rdiag + 1) * SK,
                                        ],
                                        et[
                                            :,
                                            hh * TQ
                                            + rdiag * SK : hh * TQ
                                            + (rdiag + 1) * SK,
                                        ],
                                        mask_sb[:],
                                        op=mybir.AluOpType.mult,
                                    )
                            # filler: spread proj(tcq+1) evenly over the
                            # window; O units fill whatever room is left
                            want = 2
                            if proj_left > 0:
                                want = max(
                                    2, -(-proj_left // steps_left)  # ceil div
                                )
                                got = _pull(proj_gen, want)
                                proj_left -= got
                                if got < 2:
                                    _pull(o_carry, 2 - got)
                            else:
                                _pull(o_carry, want)
                            steps_left -= 1
                            if prev is not None:
                                _z_mms(*prev)
                            prev = (j, et, rdiag)
                        _z_mms(*prev)

                        # normalize: zT = zT_unnorm * (1/denom)
                        for hh in range(2):
                            dn = rcp_pool.tile([1, TQ], f32, tag="dn", name="dn")
                            nc.vector.tensor_copy(dn[:], pz[hh][DK : DK + 1, :])
                            rcp = rcp_pool.tile([1, TQ], f32, tag="rcp", name="rcp")
                            # custom-DVE op: SBUF->SBUF only (PSUM src
                            # diverges on HW)
                            nc.vector.reciprocal_approx_fast(rcp[:], dn[:])
                            rb_sb = rbs_pool.tile([DK, TQ], f32, tag="rbs", name="rbs")
                            nc.gpsimd.partition_broadcast(rb_sb[:], rcp[:])
                            with nc.allow_low_precision(reason="bf16 z"):
                                nc.vector.tensor_mul(
                                    zT_sb[p][hh * DK : (hh + 1) * DK, sl],
                                    pz[hh][0:DK, :],
                                    rb_sb[:],
                                )

                    # proj(tcq+1) must be fully issued before attn(tcq+1)
                    for g in proj_gen:
                        _exhaust(g)

                for g in o_carry:
                    _exhaust(g)
                _exhaust(o_units(TC - 1, drain_engine="scalar"))

    nc.compile()
    return nc


def _get_program(t=T, d=D, reps=1):
    key = (t, d, reps)
    if key not in _PROGRAM_CACHE:
        _PROGRAM_CACHE[key] = _build_program(t, d, reps)
    return _PROGRAM_CACHE[key]


def _bf16(a):
    import ml_dtypes

    return np.ascontiguousarray(a).astype(ml_dtypes.bfloat16)


def _make_masks():
    # lower-triangular keep-mask for the exact diagonal 128x128 block
    i = np.arange(SK)[:, None]
    j = np.arange(SK)[None, :]
    return (i <= j).astype(np.float32)  # [128, 128]


def _core_inputs(x, Qs, Qbs, Ks, Kbs, Vs, Vbs, O, b, g, mask_host):
    hs = slice(H * g, H * (g + 1))
    xT_b = np.ascontiguousarray(x[b].T)  # [D, T]
    wq_g = np.ascontiguousarray(Qs[hs].transpose(1, 0, 2).reshape(D, H * DK))
    wk_g = np.ascontiguousarray(Ks[hs].transpose(1, 0, 2).reshape(D, H * DK))
    wv_g = np.ascontiguousarray(Vs[hs].transpose(1, 0, 2).reshape(D, H * DK))
    wo_g = np.ascontiguousarray(O[hs].reshape(H * DK, D))
    qb_flat = Qbs[hs].reshape(H * DK)
    kb_flat = Kbs[hs].reshape(H * DK)
    qb_g = np.ascontiguousarray(np.stack([qb_flat[0:128], qb_flat[128:256]], axis=1))
    kb_g = np.ascontiguousarray(np.stack([kb_flat[0:128], kb_flat[128:256]], axis=1))
    vb_g = np.ascontiguousarray(Vbs[hs].reshape(1, H * DK))
    return {
        "xT": _bf16(xT_b),
        "wq": _bf16(wq_g),
        "wk": _bf16(wk_g),
        "wv": _bf16(wv_g),
        "wo": _bf16(wo_g),
        "qb": qb_g.astype(np.float32),
        "kb": kb_g.astype(np.float32),
        "vb": _bf16(vb_g),
        "mask": _bf16(mask_host),
    }


def _build_in_maps(x, Qs, Qbs, Ks, Kbs, Vs, Vbs, O_):
    mask_host = _make_masks()
    in_maps = []
    for core in range(NCORES):
        b, g = divmod(core, NH // H)
        in_maps.append(_core_inputs(x, Qs, Qbs, Ks, Kbs, Vs, Vbs, O_, b, g, mask_host))
    return in_maps


def kernel(normalized_resid_pre, Qs, Qbs, Ks, Kbs, Vs, Vbs, O, Ob):
    from concourse.bass_utils import run_bass_kernel_spmd

    x = np.asarray(normalized_resid_pre, dtype=np.float32)
    Qs, Qbs = np.asarray(Qs, np.float32), np.asarray(Qbs, np.float32)
    Ks, Kbs = np.asarray(Ks, np.float32), np.asarray(Kbs, np.float32)
    Vs, Vbs = np.asarray(Vs, np.float32), np.asarray(Vbs, np.float32)
    O_, Ob = np.asarray(O, np.float32), np.asarray(Ob, np.float32)

    nc = _get_program()
    in_maps = _build_in_maps(x, Qs, Qbs, Ks, Kbs, Vs, Vbs, O_)

    res = run_bass_kernel_spmd(
        nc, in_maps, core_ids=list(range(NCORES)), trace=TRACE, **TRACE_KWARGS
    )
    kernel.last_results = res

    out = np.zeros((B, T, D), dtype=np.float32)
    for core in range(NCORES):
        b, g = divmod(core, NH // H)
        out[b] += res.results[core]["out"]
    out += Ob[None, None, :]
    return out
